# revision 5
# baseline (speedup 1.0000x reference)
"""Trainium2 Bass kernel for 2-layer GATv2 (nn_GCNAttn_1494648619259).

Contract: kernel(**inputs) takes FULL unsharded inputs (numpy), returns the
FULL output [B, 128, N] float32. Internally shards across 8 NeuronCores:
2 graphs (batch) x 4 destination-node shards, one launch per GNN layer.

Self-contained: hardcodes all shapes; no sibling imports.

Math (per layer):
  xl = feat @ Wl + bl ; xr = feat @ Wr + br          (dense "tables", bf16)
  per edge (s,d):  e = xl[s] + xr[d];  score_h = sum_c att[h,c]*lrelu(e)
  alpha = softmax_over_d(score);  out[d] = sum alpha * xl[s]  (per head)

The datapath is bf16 (tables, gathers, elementwise, PE matmuls) with fp32
scores/softmax/PSUM accumulation; simulated end-to-end rel err vs fp64 is
3.7e-3 (gate is 2e-2). LeakyReLU runs directly on the scalar engine
(Lrelu, alpha=0.1); attention weights are applied as an elementwise mult
before the score reduce, so no table pre-scaling/permutation is needed.

Per dst tile: two SWDGE gathers fetch XL[src] (global table) and XR[dst]
(core-local table: only this core's 40 dst tiles, built from the featTR
input so one program serves all cores). The softmax denominator is fused
into the scatter matmul: exp scores ride as 2 extra columns of the message
matrix G, so S^T @ G yields [weighted sums | denom] in one PSUM chain.

Sharding: nodes are dealt round-robin by degree into 160 tiles of 128 slots
(125 real + 3 dummy); edges grouped by dst tile, padded to T*128 slots.
Core c handles tiles [40*c4, 40*(c4+1)) of graph c//4 (c4 = c % 4).
"""
import numpy as np
import ml_dtypes
from contextlib import ExitStack

import concourse.bass as bass
import concourse.mybir as mybir
import concourse.tile as tile
from concourse import bacc
from concourse.bass_utils import run_bass_kernel_spmd

# ---- problem constants ----
H = 2
C = 64
F = 2 * C          # 128
NEG = 0.1
N = 20000
Bn = 2
F_IN = 32
NT_TOTAL = 160
P = 128
NPAD = NT_TOTAL * P     # 20480
NT_CORE = 40
NCORES = 8
BT = 8                  # tiles per batched DMA in table/out phases

BF = mybir.dt.bfloat16
F32 = mybir.dt.float32
NPBF = ml_dtypes.bfloat16


# ======================= host-side planning =======================

def _build_graph_plan(edge_index):
    src0 = edge_index[0].astype(np.int64)
    dst0 = edge_index[1].astype(np.int64)
    loops = np.arange(N, dtype=np.int64)
    src = np.concatenate([src0, loops])
    dst = np.concatenate([dst0, loops])

    deg = np.bincount(dst, minlength=N)
    order = np.argsort(-deg, kind="stable")
    node_ids = -np.ones((NT_TOTAL, P), dtype=np.int64)
    nfull = (N // NT_TOTAL)  # 125
    # deal round-robin: chunk s -> slot s across all tiles
    for s in range(nfull):
        node_ids[:, s] = order[s * NT_TOTAL:(s + 1) * NT_TOTAL]
    rem = order[nfull * NT_TOTAL:]
    if len(rem):
        node_ids[:len(rem), nfull] = rem
    node_tile = np.zeros(N, dtype=np.int64)
    node_slot = np.zeros(N, dtype=np.int64)
    for t in range(NT_TOTAL):
        ids = node_ids[t]
        v = ids >= 0
        node_tile[ids[v]] = t
        node_slot[ids[v]] = np.nonzero(v)[0]
    node_pos = node_tile * P + node_slot

    etile = node_tile[dst]
    eorder = np.argsort(etile, kind="stable")
    src_s, dst_s, et_s = src[eorder], dst[eorder], etile[eorder]
    counts = np.bincount(et_s, minlength=NT_TOTAL)
    T = int(np.ceil(counts.max() / P))
    ES = T * P

    src_pad = np.zeros((NT_TOTAL, ES), dtype=np.int64)
    dst_pad = np.zeros((NT_TOTAL, ES), dtype=np.int64)
    dstloc = -np.ones((NT_TOTAL, ES), dtype=np.float32)
    off = 0
    for t in range(NT_TOTAL):
        c = counts[t]
        src_pad[t, :c] = node_pos[src_s[off:off + c]]
        dst_pad[t, :c] = node_pos[dst_s[off:off + c]]
        dstloc[t, :c] = node_slot[dst_s[off:off + c]].astype(np.float32)
        off += c
    return dict(node_ids=node_ids, node_pos=node_pos, T=T,
                src_pad=src_pad, dst_pad=dst_pad, dstloc=dstloc)


def _wrap_idx16(idx, es):
    """[ES] int -> wrapped [128, ES//16] int16: idx i at [i%16, i//16],
    replicated across the 8 GpSimd sub-cores (partitions 16k..16k+15)."""
    a = idx.astype(np.int16).reshape(es // 16, 16).T  # [16, ES/16]
    return np.tile(a, (8, 1))


def _bcast_rows(v, dt):
    """[F] -> [128, F] replicated."""
    return np.tile(np.asarray(v, np.float32)[None, :], (P, 1)).astype(dt)


# ======================= bass kernel build =======================

def _build_layer_program(nc, T, K, odt):
    """Emit one layer's program. K = input feature count (32 or 128).

    ExternalInputs (per core):
      featT   [K, NPAD]        bf16  all tiles' features (tiled node order)
      featTR  [K, NT_CORE*P]   bf16  this core's own tiles' features
      Wl, Wr  [K, F]           bf16
      bl_b, br_b, att_b  [128, F] bf16 broadcast consts
      bias_b  [128, F]   f32
      iota_b  [128, 128] bf16 (row 0..127 in every partition)
      dloc    [128, NT_CORE*T] bf16 local dst slot per edge slot (-1 pad)
      sidx    [128, NT_CORE*ES/16] int16 wrapped gather idxs (src, global)
      didx    [128, NT_CORE*ES/16] int16 wrapped gather idxs (dst, local)
    ExternalOutput:
      hout [NT_CORE*128, F] odt
    """
    ES = T * P
    IW = ES // 16  # idx cols per tile

    featT = nc.dram_tensor("featT", [K, NPAD], BF, kind="ExternalInput").ap()
    featTR = nc.dram_tensor("featTR", [K, NT_CORE * P], BF,
                            kind="ExternalInput").ap()
    Wl = nc.dram_tensor("Wl", [K, F], BF, kind="ExternalInput").ap()
    Wr = nc.dram_tensor("Wr", [K, F], BF, kind="ExternalInput").ap()
    bl_b = nc.dram_tensor("bl_b", [P, F], BF, kind="ExternalInput").ap()
    br_b = nc.dram_tensor("br_b", [P, F], BF, kind="ExternalInput").ap()
    att_b = nc.dram_tensor("att_b", [P, F], BF, kind="ExternalInput").ap()
    bias_b = nc.dram_tensor("bias_b", [P, F], F32, kind="ExternalInput").ap()
    iota_in = nc.dram_tensor("iota_b", [P, P], BF, kind="ExternalInput").ap()
    dloc_in = nc.dram_tensor("dloc", [P, NT_CORE * T], BF,
                             kind="ExternalInput").ap()
    sidx_in = nc.dram_tensor("sidx", [P, NT_CORE * IW], mybir.dt.int16,
                             kind="ExternalInput").ap()
    didx_in = nc.dram_tensor("didx", [P, NT_CORE * IW], mybir.dt.int16,
                             kind="ExternalInput").ap()
    hout = nc.dram_tensor("hout", [NT_CORE * P, F], odt,
                          kind="ExternalOutput").ap()

    with tile.TileContext(nc) as tc, ExitStack() as ctx:
        const = ctx.enter_context(tc.tile_pool(name="const", bufs=1))
        dram = ctx.enter_context(tc.tile_pool(name="dram", bufs=1,
                                              space="DRAM"))

        # ---- resident constants ----
        wl_sb = const.tile([K, F], BF)
        nc.sync.dma_start(wl_sb[:], Wl[:])
        wr_sb = const.tile([K, F], BF)
        nc.sync.dma_start(wr_sb[:], Wr[:])
        bl_sb = const.tile([P, F], BF)
        nc.sync.dma_start(bl_sb[:], bl_b[:])
        br_sb = const.tile([P, F], BF)
        nc.sync.dma_start(br_sb[:], br_b[:])
        att_sb = const.tile([P, F], BF)
        nc.sync.dma_start(att_sb[:], att_b[:])
        bias_sb = const.tile([P, F], F32)
        nc.sync.dma_start(bias_sb[:], bias_b[:])
        iota_sb = const.tile([P, P], BF)
        nc.sync.dma_start(iota_sb[:], iota_in[:])
        dloc_sb = const.tile([P, NT_CORE * T], BF)
        nc.sync.dma_start(dloc_sb[:], dloc_in[:])
        sidx_sb = const.tile([P, NT_CORE * IW], mybir.dt.int16)
        nc.sync.dma_start(sidx_sb[:], sidx_in[:])
        didx_sb = const.tile([P, NT_CORE * IW], mybir.dt.int16)
        nc.sync.dma_start(didx_sb[:], didx_in[:])

        # ---- table phase: XL [NPAD, F] bf16 (all tiles);
        #      XR [NT_CORE*P, F] bf16 (own tiles, from featTR) ----
        xla = dram.tile([NPAD, F], BF)
        xra = dram.tile([NT_CORE * P, F], BF)
        with tc.tile_pool(name="tab", bufs=3) as tab, \
                tc.tile_pool(name="tps", bufs=2, space="PSUM") as tps:
            for b in range(NT_TOTAL // BT):
                ft = tab.tile([K, BT * P], BF, tag="ft")
                nc.sync.dma_start(ft[:], featT[:, b * BT * P:(b + 1) * BT * P])
                ot = tab.tile([P, BT, F], BF, tag="ot")
                for q in range(BT):
                    pl = tps.tile([P, F], F32, tag="pl", space="PSUM")
                    nc.tensor.matmul(pl[:], ft[:, q * P:(q + 1) * P],
                                     wl_sb[:], start=True, stop=True)
                    nc.vector.tensor_tensor(out=ot[:, q, :], in0=pl[:],
                                            in1=bl_sb[:],
                                            op=mybir.AluOpType.add)
                nc.sync.dma_start(
                    xla[b * BT * P:(b + 1) * BT * P, :]
                    .rearrange("(q p) f -> p q f", p=P), ot[:])
            for b in range(NT_CORE // BT):
                ftr = tab.tile([K, BT * P], BF, tag="ftr")
                nc.sync.dma_start(ftr[:],
                                  featTR[:, b * BT * P:(b + 1) * BT * P])
                ot2 = tab.tile([P, BT, F], BF, tag="ot2")
                for q in range(BT):
                    pr = tps.tile([P, F], F32, tag="pr", space="PSUM")
                    nc.tensor.matmul(pr[:], ftr[:, q * P:(q + 1) * P],
                                     wr_sb[:], start=True, stop=True)
                    nc.vector.tensor_tensor(out=ot2[:, q, :], in0=pr[:],
                                            in1=br_sb[:],
                                            op=mybir.AluOpType.add)
                nc.sync.dma_start(
                    xra[b * BT * P:(b + 1) * BT * P, :]
                    .rearrange("(q p) f -> p q f", p=P), ot2[:])

        # ---- edge phase ----
        gath = ctx.enter_context(tc.tile_pool(name="gath", bufs=4))
        work = ctx.enter_context(tc.tile_pool(name="work", bufs=3))
        ops = ctx.enter_context(tc.tile_pool(name="ops", bufs=4,
                                             space="PSUM"))
        outp = ctx.enter_context(tc.tile_pool(name="outp", bufs=2))
        hbp = ctx.enter_context(tc.tile_pool(name="hbp", bufs=2))

        hb = None
        for t in range(NT_CORE):
            a_g = gath.tile([P, T, F], BF, tag="a")      # XL[src]
            nc.gpsimd.dma_gather(
                out_ap=a_g[:], in_ap=xla[:],
                idxs_ap=sidx_sb[:, t * IW:(t + 1) * IW],
                num_idxs=ES, num_idxs_reg=ES, elem_size=F,
                single_packet=False, queue_num=(2 * t) % 4)
            b_g = gath.tile([P, T, F], BF, tag="b")      # XR[dst] (local)
            nc.gpsimd.dma_gather(
                out_ap=b_g[:], in_ap=xra[:],
                idxs_ap=didx_sb[:, t * IW:(t + 1) * IW],
                num_idxs=ES, num_idxs_reg=ES, elem_size=F,
                single_packet=False, queue_num=(2 * t + 1) % 4)

            eatt = work.tile([P, T, F], BF, tag="eatt")
            nc.vector.tensor_tensor(out=eatt[:], in0=a_g[:], in1=b_g[:],
                                    op=mybir.AluOpType.add)
            le = work.tile([P, T, F], BF, tag="le")
            nc.scalar.activation(le[:], eatt[:],
                                 mybir.ActivationFunctionType.Prelu,
                                 alpha=NEG)
            wle = work.tile([P, T, F], BF, tag="wle")
            nc.vector.tensor_tensor(
                out=wle[:], in0=le[:],
                in1=att_sb[:].unsqueeze(1).to_broadcast([P, T, F]),
                op=mybir.AluOpType.mult)
            sc = work.tile([P, T, H], F32, tag="sc")
            nc.vector.tensor_reduce(
                out=sc[:], in_=wle[:].rearrange("p t (h c) -> p t h c", h=H),
                axis=mybir.AxisListType.X, op=mybir.AluOpType.add)
            ex = work.tile([P, T, H], F32, tag="ex")
            nc.scalar.activation(ex[:], sc[:],
                                 mybir.ActivationFunctionType.Exp)

            # G = [exp-weighted messages | exp scores]
            G = work.tile([P, T, F + H], BF, tag="G")
            nc.vector.tensor_tensor(
                out=G[:, :, 0:F].rearrange("p t (h c) -> p t h c", h=H),
                in0=a_g[:].rearrange("p t (h c) -> p t h c", h=H),
                in1=ex[:].unsqueeze(3).to_broadcast([P, T, H, C]),
                op=mybir.AluOpType.mult)
            nc.scalar.activation(G[:, :, F:F + H], ex[:],
                                 mybir.ActivationFunctionType.Copy)

            # S[e, d] one-hot of local dst slot
            S = work.tile([P, T, P], BF, tag="S")
            dl = dloc_sb[:, t * T:(t + 1) * T]
            nc.vector.tensor_tensor(
                out=S[:],
                in0=dl.unsqueeze(2).to_broadcast([P, T, P]),
                in1=iota_sb[:].unsqueeze(1).to_broadcast([P, T, P]),
                op=mybir.AluOpType.is_equal)

            acc = ops.tile([P, F + H], F32, tag="acc", space="PSUM")
            for tt in range(T):
                nc.tensor.matmul(acc[:], S[:, tt, :], G[:, tt, :],
                                 start=(tt == 0), stop=(tt == T - 1))

            # epilogue: out = acc[:, :F] / denom + bias
            den = outp.tile([P, H], F32, tag="den")
            nc.vector.tensor_scalar(
                out=den[:], in0=acc[:, F:F + H], scalar1=1e-30, scalar2=None,
                op0=mybir.AluOpType.add)
            rec = outp.tile([P, H], F32, tag="rec")
            nc.vector.reciprocal(rec[:], den[:])
            hq = outp.tile([P, H, C], F32, tag="hq")
            nc.vector.tensor_tensor(
                out=hq[:],
                in0=acc[:, 0:F].rearrange("p (h c) -> p h c", h=H),
                in1=rec[:].unsqueeze(2).to_broadcast([P, H, C]),
                op=mybir.AluOpType.mult)
            if t % BT == 0:
                hb = hbp.tile([P, BT, F], odt, tag="hb")
            nc.vector.tensor_tensor(
                out=hb[:, t % BT, :],
                in0=hq[:].rearrange("p h c -> p (h c)"),
                in1=bias_sb[:], op=mybir.AluOpType.add)
            if t % BT == BT - 1:
                b0 = (t - BT + 1) * P
                nc.sync.dma_start(
                    hout[b0:b0 + BT * P, :]
                    .rearrange("(q p) f -> p q f", p=P), hb[:])
    return nc


def _compile_layer(T, K, odt):
    nc = bacc.Bacc("TRN2", target_bir_lowering=False, debug=False,
                   enable_asserts=False, num_devices=NCORES,
                   num_swdge_queues=4)
    _build_layer_program(nc, T, K, odt)
    nc.compile()
    return nc


# ======================= top-level kernel =======================

def _make_core_inputs(plan, lc, featT_all, K):
    """Per-core input dicts for one layer. featT_all: [Bn][K, NPAD] bf16."""
    T = plan["T"]
    ES = T * P
    IW = ES // 16
    iota = np.tile(np.arange(P, dtype=np.float32)[None, :], (P, 1))
    in_maps = []
    for core in range(NCORES):
        g = core // 4
        c4 = core % 4
        t0 = c4 * NT_CORE
        tiles = range(t0, t0 + NT_CORE)
        sidx = np.zeros((P, NT_CORE * IW), np.int16)
        didx = np.zeros((P, NT_CORE * IW), np.int16)
        dloc = np.zeros((P, NT_CORE * T), np.float32)
        for j, t in enumerate(tiles):
            sidx[:, j * IW:(j + 1) * IW] = _wrap_idx16(plan["src_pad"][t], ES)
            # pad slots have dst_pad=0 -> negative local idx; clip to 0 so
            # the gather reads a real (finite) row instead of skipping and
            # leaving uninitialized SBUF that poisons 0*NaN in the matmul.
            didx[:, j * IW:(j + 1) * IW] = _wrap_idx16(
                np.maximum(plan["dst_pad"][t] - t0 * P, 0), ES)
            # dstloc: edge slot e = tt*128+p -> [p, tt]
            dloc[:, j * T:(j + 1) * T] = plan["dstloc"][t].reshape(T, P).T
        ftg = np.ascontiguousarray(featT_all[g])
        in_maps.append({
            "featT": ftg,
            "featTR": np.ascontiguousarray(
                ftg[:, t0 * P:(t0 + NT_CORE) * P]),
            "Wl": lc["Wl"], "Wr": lc["Wr"],
            "bl_b": lc["bl_b"], "br_b": lc["br_b"], "att_b": lc["att_b"],
            "bias_b": lc["bias_b"],
            "iota_b": iota.astype(NPBF),
            "dloc": dloc.astype(NPBF),
            "sidx": sidx, "didx": didx,
        })
    return in_maps


def _layer_consts(W_l, b_l, W_r, b_r, att, bias):
    att_flat = np.asarray(att, np.float32).reshape(-1)  # [H*C], col h*C+c
    return dict(
        Wl=np.asarray(W_l, np.float32).astype(NPBF),
        Wr=np.asarray(W_r, np.float32).astype(NPBF),
        bl_b=_bcast_rows(b_l, NPBF), br_b=_bcast_rows(b_r, NPBF),
        att_b=_bcast_rows(att_flat, NPBF),
        bias_b=_bcast_rows(bias, np.float32),
    )


_RESULTS_LOG = {}


def kernel(x, edge_index, Wl1, bl1, Wr1, br1, att1, bias1,
           Wl2, bl2, Wr2, br2, att2, bias2):
    x = np.asarray(x, np.float32)
    edge_index = np.asarray(edge_index)
    plan = _build_graph_plan(edge_index)
    T = plan["T"]
    lc1 = _layer_consts(Wl1, bl1, Wr1, br1, att1, bias1)
    lc2 = _layer_consts(Wl2, bl2, Wr2, br2, att2, bias2)

    # layer 1 inputs: featT = x[g] scattered to tiled node order
    featT1 = []
    for g in range(Bn):
        ft = np.zeros((F_IN, NPAD), np.float32)
        ft[:, plan["node_pos"]] = x[g]
        featT1.append(ft.astype(NPBF))

    nc1 = _compile_layer(T, F_IN, BF)
    maps1 = _make_core_inputs(plan, lc1, featT1, F_IN)
    res1 = run_bass_kernel_spmd(nc1, maps1, list(range(NCORES)))
    _RESULTS_LOG["l1"] = res1

    # assemble h1 bf16 [F, NPAD] per graph, zero dummy rows
    mask = (plan["node_ids"].reshape(-1) >= 0)
    featT2 = []
    for g in range(Bn):
        parts = [res1.results[g * 4 + c4]["hout"] for c4 in range(4)]
        h = np.concatenate(parts, axis=0)  # [NPAD, F] bf16
        h[~mask] = 0
        featT2.append(np.ascontiguousarray(h.T))  # [F, NPAD] bf16

    nc2 = _compile_layer(T, F, F32)
    maps2 = _make_core_inputs(plan, lc2, featT2, F)
    res2 = run_bass_kernel_spmd(nc2, maps2, list(range(NCORES)))
    _RESULTS_LOG["l2"] = res2

    out = np.zeros((Bn, F, N), np.float32)
    for g in range(Bn):
        parts = [res2.results[g * 4 + c4]["hout"] for c4 in range(4)]
        h2p = np.concatenate(parts, axis=0).astype(np.float32)  # [NPAD, F]
        out[g] = h2p[plan["node_pos"]].T
    return out
